# revision 15
# baseline (speedup 1.0000x reference)
"""Trainium2 Bass kernel for nn_ExplorationBehavior (scatter_memory).

Strategy (8 NeuronCores, SPMD):
  - Batch-shard the policy MLP / place cells / novelty (4096 rows per core).
  - Scatter algebra: since the reference gathers `old` from the ORIGINAL map,
      new_map = (1 - ALPHA*n) * map + ALPHA * S
    where n[cell] = #items in cell, S[cell,:] = sum of brain_state rows in cell.
  - Each core computes S,n over its own batch shard via matmul-based binning:
    10 bins of 256 cells; per-bin member lists built with a one-hot(rank) x
    one-hot(bin) compaction matmul; rows fetched via indirect-DMA gathers;
    per-bin one-hot(cell) matmuls accumulate S in PSUM.
  - One fp16 ReduceScatter of [2560,1025] (= alpha*S || n) combines shards;
    each core finalizes 320 cells of new_map / new_visit.

kernel(**inputs) takes FULL inputs, shards internally, returns FULL outputs.
"""

import numpy as np

import concourse.bacc as bacc
import concourse.bass as bass
import concourse.mybir as mybir
import concourse.tile as tile
from concourse.bass_utils import run_bass_kernel_spmd
from concourse.masks import make_identity, make_upper_triangular

F32 = mybir.dt.float32
F16 = mybir.dt.float16
I32 = mybir.dt.int32
OP = mybir.AluOpType
AF = mybir.ActivationFunctionType
AX = mybir.AxisListType

NCORES = 8
B = 32768
BSH = B // NCORES          # 4096 rows per core
D = 1024
PC = 100                   # place cells
M = 50                     # map resolution
H = 256                    # hidden
IN = D + PC                # 1124
ALPHA = 0.1

NB = 10                    # bins (cell // 256)
CPB = 256                  # cells per bin
KLOC = 512                 # per-core per-bin slot capacity (mean 409.6)
NT = BSH // 128            # 32 batch tiles per core
WC = BSH // 128            # 32 wide columns ([128, 32] wide layout)
CELLS_PAD = 2560           # 20 * 128
CSLICE = CELLS_PAD // NCORES  # 320 cells per core after ReduceScatter
GRP = 4                    # MLP batch-tiles per weight-stationary group
DEBUG = False


def _floor_exact(nc, pool, x, tag):
    """Exact floor for non-negative fp32 (HW cast rounds-to-nearest)."""
    p, f = x.shape
    xi = pool.tile([p, f], I32, tag=tag + "_i")
    xf = pool.tile([p, f], F32, tag=tag + "_f")
    gt = pool.tile([p, f], F32, tag=tag + "_g")
    nc.vector.tensor_copy(out=xi[:], in_=x)
    nc.vector.tensor_copy(out=xf[:], in_=xi[:])
    nc.vector.tensor_tensor(out=gt[:], in0=xf[:], in1=x, op=OP.is_gt)
    nc.vector.tensor_tensor(out=xf[:], in0=xf[:], in1=gt[:], op=OP.subtract)
    return xf


def build_kernel():
    nc = bacc.Bacc("TRN2", target_bir_lowering=False, debug=False,
                   num_devices=NCORES)

    # ---------------- DRAM I/O ----------------
    brain_sh = nc.dram_tensor("brain_sh", [BSH, D], F32, kind="ExternalInput")
    pos_sh = nc.dram_tensor("pos_sh", [BSH, 2], F32, kind="ExternalInput")
    centers = nc.dram_tensor("centers", [PC, 2], F32, kind="ExternalInput")
    widths = nc.dram_tensor("widths", [PC], F32, kind="ExternalInput")
    visit_full = nc.dram_tensor("visit_full", [M * M], F32, kind="ExternalInput")
    map_slice = nc.dram_tensor("map_slice", [CSLICE, D], F32, kind="ExternalInput")
    visit_slice = nc.dram_tensor("visit_slice", [CSLICE, 1], F32, kind="ExternalInput")
    w1 = nc.dram_tensor("w1", [IN, H], F32, kind="ExternalInput")
    b1 = nc.dram_tensor("b1", [H], F32, kind="ExternalInput")
    w2 = nc.dram_tensor("w2", [H, H], F32, kind="ExternalInput")
    b2 = nc.dram_tensor("b2", [H], F32, kind="ExternalInput")
    wd = nc.dram_tensor("wd", [H, 9], F32, kind="ExternalInput")
    bd = nc.dram_tensor("bd", [9], F32, kind="ExternalInput")
    ws = nc.dram_tensor("ws", [H, 1], F32, kind="ExternalInput")
    bs = nc.dram_tensor("bs", [1], F32, kind="ExternalInput")
    wg = nc.dram_tensor("wg", [H, 1], F32, kind="ExternalInput")
    bg = nc.dram_tensor("bg", [1], F32, kind="ExternalInput")

    dir_out = nc.dram_tensor("dir_out", [BSH, 9], F32, kind="ExternalOutput")
    speed_out = nc.dram_tensor("speed_out", [BSH, 1], F32, kind="ExternalOutput")
    expl_out = nc.dram_tensor("expl_out", [BSH, 1], F32, kind="ExternalOutput")
    nov_out = nc.dram_tensor("nov_out", [BSH, 1], F32, kind="ExternalOutput")
    pc_out = nc.dram_tensor("pc_out", [BSH, PC], F32, kind="ExternalOutput")
    map_out = nc.dram_tensor("map_out", [CSLICE, D], F32, kind="ExternalOutput")
    visit_out = nc.dram_tensor("visit_out", [CSLICE, 1], F32, kind="ExternalOutput")

    if DEBUG:
        dbg_lrank = nc.dram_tensor("dbg_lrank", [128, WC], F32, kind="ExternalOutput")
        dbg_binw = nc.dram_tensor("dbg_binw", [128, WC], F32, kind="ExternalOutput")
        dbg_segT = nc.dram_tensor("dbg_segT", [128, 4 * NB], F32, kind="ExternalOutput")
        dbg_rsin = nc.dram_tensor("dbg_rsin", [CELLS_PAD, D + 1], F16, kind="ExternalOutput")
        dbg_xt = nc.dram_tensor("dbg_xt", [128, 9 * 128], F32, kind="ExternalOutput")
        dbg_h1 = nc.dram_tensor("dbg_h1", [128, 1024], F32, kind="ExternalOutput")
        dbg_h2 = nc.dram_tensor("dbg_h2", [128, 1024], F32, kind="ExternalOutput")
        dbg_hd = nc.dram_tensor("dbg_hd", [128, 11], F32, kind="ExternalOutput")

    with tile.TileContext(nc) as tc:
        with (
            tc.tile_pool(name="const", bufs=1) as cst,
            tc.tile_pool(name="wide", bufs=1) as wd_p,
            tc.tile_pool(name="work", bufs=2) as wk,
            tc.tile_pool(name="mlp", bufs=2) as mp,
            tc.tile_pool(name="xt", bufs=5) as xtp,
            tc.tile_pool(name="gb", bufs=6) as gbp,
            tc.tile_pool(name="ps", bufs=2, space="PSUM") as ps,
            tc.tile_pool(name="psS", bufs=2, space="PSUM") as psS,
            tc.tile_pool(name="dram", bufs=1, space="DRAM") as dr,
        ):
            # ============ constants ============
            ident = cst.tile([128, 128], F32)
            make_identity(nc, ident[:])
            ltri = cst.tile([128, 128], F32)   # ltri[k,i]=1 iff k<i
            make_upper_triangular(nc, ltri[:], val=1.0, diag=False)

            iota10 = cst.tile([128, NB], I32)
            nc.gpsimd.iota(iota10[:], pattern=[[1, NB]], base=0, channel_multiplier=0)
            iota10f = cst.tile([128, NB], F32)
            nc.vector.tensor_copy(out=iota10f[:], in_=iota10[:])

            iota512 = cst.tile([128, KLOC], I32)
            nc.gpsimd.iota(iota512[:], pattern=[[1, KLOC]], base=0, channel_multiplier=0)
            iota512f = cst.tile([128, KLOC], F32)
            nc.vector.tensor_copy(out=iota512f[:], in_=iota512[:])

            iotaA = cst.tile([128, 128], I32)   # 0..127 along free
            nc.gpsimd.iota(iotaA[:], pattern=[[1, 128]], base=0, channel_multiplier=0)
            iotaAf = cst.tile([128, 128], F32)
            nc.vector.tensor_copy(out=iotaAf[:], in_=iotaA[:])
            iotaB = cst.tile([128, 128], I32)   # 128..255 along free
            nc.gpsimd.iota(iotaB[:], pattern=[[1, 128]], base=128, channel_multiplier=0)
            iotaBf = cst.tile([128, 128], F32)
            nc.vector.tensor_copy(out=iotaBf[:], in_=iotaB[:])

            iota50 = cst.tile([128, M], I32)
            nc.gpsimd.iota(iota50[:], pattern=[[1, M]], base=0, channel_multiplier=0)
            iota50f = cst.tile([128, M], F32)
            nc.vector.tensor_copy(out=iota50f[:], in_=iota50[:])

            # mem id in wide layout: item = p*WC + c
            imem = cst.tile([128, WC], I32)
            nc.gpsimd.iota(imem[:], pattern=[[1, WC]], base=0, channel_multiplier=WC)
            imemf = cst.tile([128, WC], F32)
            nc.vector.tensor_copy(out=imemf[:], in_=imem[:])

            ones1 = cst.tile([1, 128], F32)
            nc.vector.memset(ones1[:], 1.0)
            onescol = cst.tile([128, 1], F32)
            nc.vector.memset(onescol[:], 1.0)

            # visit table as [50 part, 50 free] for novelty matmul-gather
            c5050 = cst.tile([M, M], F32)
            nc.sync.dma_start(out=c5050[:], in_=visit_full[:].rearrange("(a b) -> a b", a=M))

            # place-cell constant rows + broadcasts
            cxr = cst.tile([1, PC], F32)
            cyr = cst.tile([1, PC], F32)
            wr = cst.tile([1, PC], F32)
            nc.sync.dma_start(out=cxr[:], in_=centers[:, 0:1])
            nc.sync.dma_start(out=cyr[:], in_=centers[:, 1:2])
            nc.sync.dma_start(out=wr[:], in_=widths[:].rearrange("(o a) -> o a", o=1))
            w2r = cst.tile([1, PC], F32)
            nc.scalar.activation(out=w2r[:], in_=wr[:], func=AF.Square, scale=2.0)
            # w2r = (2w)^2/2? No: Square(2*w) = 4w^2.  We want -1/(2 w^2) = -2/(4w^2).
            wrec = cst.tile([1, PC], F32)
            nc.vector.reciprocal(out=wrec[:], in_=w2r[:])
            frow = cst.tile([1, PC], F32)
            nc.vector.tensor_scalar_mul(out=frow[:], in0=wrec[:], scalar1=-2.0)

            def bcast128(row, n, tag):
                pt = ps.tile([128, n], F32, tag="gen")
                nc.tensor.matmul(out=pt[:], lhsT=ones1[:], rhs=row, start=True, stop=True)
                sb_t = cst.tile([128, n], F32, tag=tag)
                nc.vector.tensor_copy(out=sb_t[:], in_=pt[:])
                return sb_t

            cx_rep = bcast128(cxr[:], PC, "cx_rep")
            cy_rep = bcast128(cyr[:], PC, "cy_rep")
            f_rep = bcast128(frow[:], PC, "f_rep")

            # head bias row [1, 11] -> [128, 11]
            hbrow = cst.tile([1, 11], F32)
            nc.sync.dma_start(out=hbrow[:, 0:9], in_=bd[:].rearrange("(o a) -> o a", o=1))
            nc.sync.dma_start(out=hbrow[:, 9:10], in_=bs[:].rearrange("(o a) -> o a", o=1))
            nc.sync.dma_start(out=hbrow[:, 10:11], in_=bg[:].rearrange("(o a) -> o a", o=1))
            hb_rep = bcast128(hbrow[:], 11, "hb_rep")

            # weights
            w1c = []
            for k in range(9):
                t = cst.tile([128, H], F32, tag=f"w1c{k}")
                if k < 8:
                    nc.sync.dma_start(out=t[:], in_=w1[k * 128:(k + 1) * 128, :])
                else:
                    nc.vector.memset(t[:], 0.0)
                    nc.sync.dma_start(out=t[:100, :], in_=w1[1024:1124, :])
                w1c.append(t)
            w2c = [[cst.tile([128, 128], F32, tag=f"w2c{j}{m}", name=f"w2c{j}{m}")
                    for m in range(2)] for j in range(2)]
            for j in range(2):
                for m in range(2):
                    nc.sync.dma_start(out=w2c[j][m][:],
                                      in_=w2[j * 128:(j + 1) * 128, m * 128:(m + 1) * 128])
            whd = [cst.tile([128, 11], F32, tag=f"whd{m}", name=f"whd{m}") for m in range(2)]
            for m in range(2):
                nc.sync.dma_start(out=whd[m][:, 0:9], in_=wd[m * 128:(m + 1) * 128, :])
                nc.sync.dma_start(out=whd[m][:, 9:10], in_=ws[m * 128:(m + 1) * 128, :])
                nc.sync.dma_start(out=whd[m][:, 10:11], in_=wg[m * 128:(m + 1) * 128, :])
            b1c = cst.tile([128, 2], F32)
            nc.sync.dma_start(out=b1c[:], in_=b1[:].rearrange("(a p) -> p a", p=128))
            b2c = cst.tile([128, 2], F32)
            nc.sync.dma_start(out=b2c[:], in_=b2[:].rearrange("(a p) -> p a", p=128))

            # ============ pass 1: wide-layout binning ============
            posw = wd_p.tile([128, 2 * WC], F32)   # item (p,c) at [p, 2c:2c+2]
            nc.sync.dma_start(out=posw[:], in_=pos_sh[:].rearrange("(p c) x -> p (c x)", p=128))

            sc50 = wd_p.tile([128, 2 * WC], F32)
            nc.vector.tensor_scalar_mul(out=sc50[:], in0=posw[:], scalar1=float(M))
            g_xy = _floor_exact(nc, wd_p, sc50[:], "fl1")
            nc.vector.tensor_scalar_min(out=g_xy[:], in0=g_xy[:], scalar1=float(M - 1))
            gx = g_xy[:, 0::2]
            gy = g_xy[:, 1::2]

            cellw = wd_p.tile([128, WC], F32)
            nc.vector.scalar_tensor_tensor(out=cellw[:], in0=gx, scalar=float(M),
                                           in1=gy, op0=OP.mult, op1=OP.add)
            t256 = wd_p.tile([128, WC], F32)
            nc.vector.tensor_scalar_mul(out=t256[:], in0=cellw[:], scalar1=1.0 / CPB)
            binw = _floor_exact(nc, wd_p, t256[:], "fl2")
            cellrec = wd_p.tile([128, WC], F32)
            nc.vector.scalar_tensor_tensor(out=cellrec[:], in0=binw[:], scalar=-float(CPB),
                                           in1=cellw[:], op0=OP.mult, op1=OP.add)

            # per-bin masks, scans, totals
            totals = wd_p.tile([128, NB], F32)
            masks = []
            scans = []
            for g in range(NB):
                mg = wd_p.tile([128, WC], F32, tag=f"mask{g}")
                nc.vector.tensor_scalar(out=mg[:], in0=binw[:], scalar1=float(g),
                                        scalar2=None, op0=OP.is_equal)
                sg = wd_p.tile([128, WC], F32, tag=f"scan{g}")
                nc.vector.tensor_tensor_scan(out=sg[:], data0=mg[:], data1=mg[:],
                                             initial=0.0, op0=OP.add, op1=OP.bypass)
                nc.vector.tensor_copy(out=totals[:, g:g + 1], in_=sg[:, WC - 1:WC])
                masks.append(mg)
                scans.append(sg)

            carry_ps = ps.tile([128, NB], F32, tag="gen")
            nc.tensor.matmul(out=carry_ps[:], lhsT=ltri[:], rhs=totals[:],
                             start=True, stop=True)
            carrym1 = wd_p.tile([128, NB], F32)
            nc.vector.tensor_scalar_add(out=carrym1[:], in0=carry_ps[:], scalar1=-1.0)

            lrank = wd_p.tile([128, WC], F32)
            nc.vector.memset(lrank[:], 0.0)
            for g in range(NB):
                tmp = wd_p.tile([128, WC], F32, tag="ranktmp")
                nc.vector.scalar_tensor_tensor(out=tmp[:], in0=scans[g][:],
                                               scalar=carrym1[:, g:g + 1],
                                               in1=masks[g][:], op0=OP.add, op1=OP.mult)
                nc.vector.tensor_tensor(out=lrank[:], in0=lrank[:], in1=tmp[:], op=OP.add)
            nc.vector.tensor_scalar_min(out=lrank[:], in0=lrank[:], scalar1=float(KLOC - 1))
            if DEBUG:
                nc.sync.dma_start(out=dbg_lrank[:], in_=lrank[:])
                nc.sync.dma_start(out=dbg_binw[:], in_=binw[:])

            # pack = mem*512 + cellrec + 1   (< 2^21, exact in fp32)
            packw = wd_p.tile([128, WC], F32)
            nc.vector.scalar_tensor_tensor(out=packw[:], in0=imemf[:], scalar=float(KLOC),
                                           in1=cellrec[:], op0=OP.mult, op1=OP.add)
            nc.vector.tensor_scalar_add(out=packw[:], in0=packw[:], scalar1=1.0)

            # compaction matmuls -> 4 separate psum tiles (one region each;
            # interleaved accumulation regions inside ONE psum tile are broken)
            seg_ps = [ps.tile([128, NB], F32, tag=f"acc{jc}", name=f"seg_ps{jc}", bufs=1)
                      for jc in range(4)]
            for c in range(WC):
                ohb = wk.tile([128, NB], F32, tag="ohb")
                nc.vector.tensor_scalar(out=ohb[:], in0=iota10f[:], scalar1=binw[:, c:c + 1],
                                        scalar2=None, op0=OP.is_equal)
                rk = wk.tile([128, KLOC], F32, tag="rk")
                nc.vector.tensor_scalar(out=rk[:], in0=iota512f[:], scalar1=lrank[:, c:c + 1],
                                        scalar2=None, op0=OP.is_equal)
                nc.vector.tensor_scalar(out=rk[:], in0=rk[:], scalar1=packw[:, c:c + 1],
                                        scalar2=None, op0=OP.mult)
                for jc in range(4):
                    nc.tensor.matmul(out=seg_ps[jc][:],
                                     lhsT=rk[:, jc * 128:(jc + 1) * 128], rhs=ohb[:],
                                     start=(c == 0), stop=(c == WC - 1))

            segT = wd_p.tile([128, 4 * NB], F32)
            for jc in range(4):
                nc.vector.tensor_copy(out=segT[:, jc * NB:(jc + 1) * NB], in_=seg_ps[jc][:])
            if DEBUG:
                nc.sync.dma_start(out=dbg_segT[:], in_=segT[:])
            # unpack: mem = floor(pack/512); cellrec = pack - 512*mem - 1
            pk9 = wd_p.tile([128, 4 * NB], F32)
            nc.vector.tensor_scalar_mul(out=pk9[:], in0=segT[:], scalar1=1.0 / KLOC)
            memf_t = _floor_exact(nc, wd_p, pk9[:], "fl3")
            crec_t = wd_p.tile([128, 4 * NB], F32)
            nc.vector.scalar_tensor_tensor(out=crec_t[:], in0=memf_t[:], scalar=-float(KLOC),
                                           in1=segT[:], op0=OP.mult, op1=OP.add)
            nc.vector.tensor_scalar_add(out=crec_t[:], in0=crec_t[:], scalar1=-1.0)
            nc.vector.tensor_scalar_min(out=memf_t[:], in0=memf_t[:], scalar1=float(BSH - 1))
            mem_i = wd_p.tile([128, 4 * NB], I32)
            nc.vector.tensor_copy(out=mem_i[:], in_=memf_t[:])

            # ============ novelty (own shard, wide layout) ============
            novw = wd_p.tile([128, WC], F32)
            for c in range(WC):
                oxc = wk.tile([128, M], F32, tag="oxc")
                nc.vector.tensor_scalar(out=oxc[:], in0=iota50f[:], scalar1=gx[:, c:c + 1],
                                        scalar2=None, op0=OP.is_equal)
                oxt_ps = ps.tile([M, 128], F32, tag="gen")
                nc.tensor.transpose(out=oxt_ps[:], in_=oxc[:], identity=ident[:])
                oxt = wk.tile([M, 128], F32, tag="oxt")
                nc.vector.tensor_copy(out=oxt[:], in_=oxt_ps[:])
                g1_ps = ps.tile([128, M], F32, tag="gen")
                nc.tensor.matmul(out=g1_ps[:], lhsT=oxt[:], rhs=c5050[:],
                                 start=True, stop=True)
                oyc = wk.tile([128, M], F32, tag="oyc")
                nc.vector.tensor_scalar(out=oyc[:], in0=iota50f[:], scalar1=gy[:, c:c + 1],
                                        scalar2=None, op0=OP.is_equal)
                prod = wk.tile([128, M], F32, tag="prod")
                nc.vector.tensor_tensor(out=prod[:], in0=g1_ps[:], in1=oyc[:], op=OP.mult)
                nc.vector.tensor_reduce(out=novw[:, c:c + 1], in_=prod[:],
                                        axis=AX.X, op=OP.add)
            nove = wd_p.tile([128, WC], F32)
            nc.scalar.activation(out=nove[:], in_=novw[:], func=AF.Exp, scale=-0.1)
            nc.sync.dma_start(out=nov_out[:].rearrange("(p c) x -> p (c x)", p=128),
                              in_=nove[:])

            # ============ MLP (batch tiles of 128, groups of 4) ============
            for grp in range(NT // GRP):
                xts = []
                for bt_ in range(GRP):
                    bt = grp * GRP + bt_
                    bx = mp.tile([128, D], F32, tag="bx")
                    nc.sync.dma_start(out=bx[:], in_=brain_sh[bt * 128:(bt + 1) * 128, :])
                    xt = xtp.tile([128, 9 * 128], F32, tag="xt")
                    for k in range(0, 8, 4):
                        tp_ps = ps.tile([128, 512], F32, tag="gen")
                        for q in range(4):
                            nc.tensor.transpose(out=tp_ps[:, q * 128:(q + 1) * 128],
                                                in_=bx[:, (k + q) * 128:(k + q + 1) * 128],
                                                identity=ident[:])
                        nc.vector.tensor_copy(out=xt[:, k * 128:(k + 4) * 128], in_=tp_ps[:])
                    # place cells for this tile
                    pt = mp.tile([128, 2], F32, tag="pt")
                    nc.sync.dma_start(out=pt[:], in_=pos_sh[bt * 128:(bt + 1) * 128, :])
                    npx = mp.tile([128, 2], F32, tag="npx")
                    nc.vector.tensor_scalar_mul(out=npx[:], in0=pt[:], scalar1=-1.0)
                    dx2 = mp.tile([128, PC], F32, tag="dx2")
                    nc.scalar.activation(out=dx2[:], in_=cx_rep[:], func=AF.Square,
                                         bias=npx[:, 0:1])
                    dy2 = mp.tile([128, PC], F32, tag="dy2")
                    nc.scalar.activation(out=dy2[:], in_=cy_rep[:], func=AF.Square,
                                         bias=npx[:, 1:2])
                    nc.vector.tensor_tensor(out=dx2[:], in0=dx2[:], in1=dy2[:], op=OP.add)
                    nc.vector.tensor_tensor(out=dx2[:], in0=dx2[:], in1=f_rep[:], op=OP.mult)
                    pcv = mp.tile([128, PC], F32, tag="pcv")
                    nc.scalar.activation(out=pcv[:], in_=dx2[:], func=AF.Exp)
                    nc.sync.dma_start(out=pc_out[bt * 128:(bt + 1) * 128, :], in_=pcv[:])
                    pc_ps = ps.tile([PC, 128], F32, tag="gen")
                    nc.tensor.transpose(out=pc_ps[:], in_=pcv[:], identity=ident[:])
                    nc.vector.tensor_copy(out=xt[:PC, 8 * 128:9 * 128], in_=pc_ps[:])
                    if DEBUG and bt == 0:
                        nc.sync.dma_start(out=dbg_xt[:], in_=xt[:])
                    xts.append(xt)

                # layer 1: h1T[j, b] per j-half; one psum tile per batch tile
                # (single accumulation region per tile -- see segT note)
                h1sb = mp.tile([128, 2 * 512], F32, tag="h1sb")
                for jh in range(2):
                    h1ps = [ps.tile([128, 128], F32, tag=f"acc{q}", name=f"h1ps{q}", bufs=1)
                            for q in range(GRP)]
                    for k in range(9):
                        for bt_ in range(GRP):
                            nc.tensor.matmul(
                                out=h1ps[bt_][:],
                                lhsT=w1c[k][:, jh * 128:(jh + 1) * 128],
                                rhs=xts[bt_][:, k * 128:(k + 1) * 128],
                                start=(k == 0), stop=(k == 8))
                    for bt_ in range(GRP):
                        nc.scalar.activation(
                            out=h1sb[:, jh * 512 + bt_ * 128:jh * 512 + (bt_ + 1) * 128],
                            in_=h1ps[bt_][:], func=AF.Relu, bias=b1c[:, jh:jh + 1])
                if DEBUG and grp == 0:
                    nc.sync.dma_start(out=dbg_h1[:], in_=h1sb[:])
                # layer 2
                h2sb = mp.tile([128, 2 * 512], F32, tag="h2sb")
                for mh in range(2):
                    h2_ps = ps.tile([128, 512], F32, tag="acc0", bufs=1)
                    for j in range(2):
                        nc.tensor.matmul(out=h2_ps[:], lhsT=w2c[j][mh],
                                         rhs=h1sb[:, j * 512:(j + 1) * 512],
                                         start=(j == 0), stop=(j == 1))
                    nc.scalar.activation(out=h2sb[:, mh * 512:(mh + 1) * 512],
                                         in_=h2_ps[:], func=AF.Relu,
                                         bias=b2c[:, mh:mh + 1])
                if DEBUG and grp == 0:
                    nc.sync.dma_start(out=dbg_h2[:], in_=h2sb[:])
                # heads
                for bt_ in range(GRP):
                    bt = grp * GRP + bt_
                    hd_ps = ps.tile([128, 11], F32, tag="gen")
                    for mh in range(2):
                        nc.tensor.matmul(out=hd_ps[:],
                                         lhsT=h2sb[:, mh * 512 + bt_ * 128:
                                                   mh * 512 + (bt_ + 1) * 128],
                                         rhs=whd[mh][:], start=(mh == 0), stop=(mh == 1))
                    hd = mp.tile([128, 11], F32, tag="hd")
                    nc.vector.tensor_tensor(out=hd[:], in0=hd_ps[:], in1=hb_rep[:], op=OP.add)
                    if DEBUG and bt == 0:
                        nc.sync.dma_start(out=dbg_hd[:], in_=hd[:])
                    # softmax over first 9
                    mx = mp.tile([128, 1], F32, tag="mx")
                    nc.vector.tensor_reduce(out=mx[:], in_=hd[:, 0:9], axis=AX.X, op=OP.max)
                    nmx = mp.tile([128, 1], F32, tag="nmx")
                    nc.vector.tensor_scalar_mul(out=nmx[:], in0=mx[:], scalar1=-1.0)
                    ex = mp.tile([128, 9], F32, tag="ex")
                    nc.scalar.activation(out=ex[:], in_=hd[:, 0:9], func=AF.Exp,
                                         bias=nmx[:])
                    sm = mp.tile([128, 1], F32, tag="sm")
                    nc.vector.tensor_reduce(out=sm[:], in_=ex[:], axis=AX.X, op=OP.add)
                    rs = mp.tile([128, 1], F32, tag="rs")
                    nc.vector.reciprocal(out=rs[:], in_=sm[:])
                    dirv = mp.tile([128, 9], F32, tag="dirv")
                    nc.vector.tensor_scalar(out=dirv[:], in0=ex[:], scalar1=rs[:],
                                            scalar2=None, op0=OP.mult)
                    nc.sync.dma_start(out=dir_out[bt * 128:(bt + 1) * 128, :], in_=dirv[:])
                    sg = mp.tile([128, 2], F32, tag="sg")
                    nc.scalar.activation(out=sg[:], in_=hd[:, 9:11], func=AF.Sigmoid)
                    nc.sync.dma_start(out=speed_out[bt * 128:(bt + 1) * 128, :], in_=sg[:, 0:1])
                    nc.sync.dma_start(out=expl_out[bt * 128:(bt + 1) * 128, :], in_=sg[:, 1:2])

            # ============ pass 2: gathers + per-bin one-hot matmuls ============
            rs_in = dr.tile([CELLS_PAD, D + 1], F16)
            rs_out = dr.tile([CSLICE, D + 1], F16)

            for g in range(NB):
                gbufs = []
                for jc in range(4):
                    col = jc * NB + g
                    gb = gbp.tile([128, D], F32, tag="gb")
                    nc.gpsimd.indirect_dma_start(
                        out=gb[:], out_offset=None, in_=brain_sh[:],
                        in_offset=bass.IndirectOffsetOnAxis(ap=mem_i[:, col:col + 1], axis=0))
                    gbufs.append(gb)
                for hh in range(2):
                    s0 = psS.tile([128, 512], F32, tag="sps")
                    s1 = psS.tile([128, 512], F32, tag="sps")
                    n_ps = ps.tile([128, 1], F32, tag="gen")
                    iot = iotaAf if hh == 0 else iotaBf
                    for jc in range(4):
                        col = jc * NB + g
                        oh = wk.tile([128, 128], F32, tag="oh")
                        nc.vector.tensor_scalar(out=oh[:], in0=iot[:],
                                                scalar1=crec_t[:, col:col + 1],
                                                scalar2=None, op0=OP.is_equal)
                        nc.tensor.matmul(out=s0[:], lhsT=oh[:], rhs=gbufs[jc][:, 0:512],
                                         start=(jc == 0), stop=(jc == 3))
                        nc.tensor.matmul(out=s1[:], lhsT=oh[:], rhs=gbufs[jc][:, 512:1024],
                                         start=(jc == 0), stop=(jc == 3))
                        nc.tensor.matmul(out=n_ps[:], lhsT=oh[:], rhs=onescol[:],
                                         start=(jc == 0), stop=(jc == 3))
                    aS = wk.tile([128, D], F16, tag="aS")
                    nc.scalar.mul(out=aS[:, 0:512], in_=s0[:], mul=ALPHA)
                    nc.scalar.mul(out=aS[:, 512:1024], in_=s1[:], mul=ALPHA)
                    nh = wk.tile([128, 1], F16, tag="nh")
                    nc.vector.tensor_copy(out=nh[:], in_=n_ps[:])
                    row0 = (g * 2 + hh) * 128
                    nc.sync.dma_start(out=rs_in[row0:row0 + 128, 0:D], in_=aS[:])
                    nc.sync.dma_start(out=rs_in[row0:row0 + 128, D:D + 1], in_=nh[:])

            # ============ ReduceScatter + final combine ============
            if DEBUG:
                nc.sync.dma_start(out=dbg_rsin[:], in_=rs_in[:])
            nc.gpsimd.collective_compute(
                "ReduceScatter", OP.add,
                replica_groups=[list(range(NCORES))],
                ins=[rs_in.opt()], outs=[rs_out.opt()])

            for ch, rows in ((0, 128), (1, 128), (2, 64)):
                r0 = ch * 128
                rsl = wk.tile([128, D + 1], F16, tag="rsl")
                nc.sync.dma_start(out=rsl[:rows, :], in_=rs_out[r0:r0 + rows, :])
                msl = wk.tile([128, D], F32, tag="msl")
                nc.sync.dma_start(out=msl[:rows, :], in_=map_slice[r0:r0 + rows, :])
                nf = wk.tile([128, 1], F32, tag="nf")
                nc.vector.tensor_copy(out=nf[:rows, :], in_=rsl[:rows, D:D + 1])
                fac = wk.tile([128, 1], F32, tag="fac")
                nc.vector.tensor_scalar(out=fac[:rows, :], in0=nf[:rows, :],
                                        scalar1=-ALPHA, scalar2=1.0,
                                        op0=OP.mult, op1=OP.add)
                aSf = wk.tile([128, D], F32, tag="aSf")
                nc.vector.tensor_copy(out=aSf[:rows, :], in_=rsl[:rows, 0:D])
                om = wk.tile([128, D], F32, tag="om")
                nc.vector.scalar_tensor_tensor(out=om[:rows, :], in0=msl[:rows, :],
                                               scalar=fac[:rows, :], in1=aSf[:rows, :],
                                               op0=OP.mult, op1=OP.add)
                nc.sync.dma_start(out=map_out[r0:r0 + rows, :], in_=om[:rows, :])
                vsl = wk.tile([128, 1], F32, tag="vsl")
                nc.sync.dma_start(out=vsl[:rows, :], in_=visit_slice[r0:r0 + rows, :])
                nc.vector.tensor_tensor(out=vsl[:rows, :], in0=vsl[:rows, :],
                                        in1=nf[:rows, :], op=OP.add)
                nc.sync.dma_start(out=visit_out[r0:r0 + rows, :], in_=vsl[:rows, :])

    nc.finalize()
    return nc


_NC_CACHE = None


def _make_in_maps(brain_state, position, place_cell_centers, place_cell_widths,
                  spatial_map, visit_count, W1, b1, W2, b2, Wd, bd, Ws, bs, Wg, bg):
    brain_state = np.asarray(brain_state, np.float32)
    position = np.asarray(position, np.float32)
    map_flat = np.asarray(spatial_map, np.float32).reshape(M * M, D)
    visit_flat = np.asarray(visit_count, np.float32).reshape(M * M)
    map_pad = np.zeros((CELLS_PAD, D), np.float32)
    map_pad[:M * M] = map_flat
    visit_pad = np.zeros((CELLS_PAD, 1), np.float32)
    visit_pad[:M * M, 0] = visit_flat

    in_maps = []
    for k in range(NCORES):
        in_maps.append({
            "brain_sh": np.ascontiguousarray(brain_state[k * BSH:(k + 1) * BSH]),
            "pos_sh": np.ascontiguousarray(position[k * BSH:(k + 1) * BSH]),
            "centers": np.asarray(place_cell_centers, np.float32),
            "widths": np.asarray(place_cell_widths, np.float32),
            "visit_full": visit_flat.copy(),
            "map_slice": np.ascontiguousarray(map_pad[k * CSLICE:(k + 1) * CSLICE]),
            "visit_slice": np.ascontiguousarray(visit_pad[k * CSLICE:(k + 1) * CSLICE]),
            "w1": np.asarray(W1, np.float32), "b1": np.asarray(b1, np.float32),
            "w2": np.asarray(W2, np.float32), "b2": np.asarray(b2, np.float32),
            "wd": np.asarray(Wd, np.float32), "bd": np.asarray(bd, np.float32),
            "ws": np.asarray(Ws, np.float32), "bs": np.asarray(bs, np.float32),
            "wg": np.asarray(Wg, np.float32), "bg": np.asarray(bg, np.float32),
        })
    return in_maps


def _gather_outputs(rs):

    direction_probs = np.concatenate([rs[k]["dir_out"] for k in range(NCORES)], 0)
    speed = np.concatenate([rs[k]["speed_out"] for k in range(NCORES)], 0)
    expl = np.concatenate([rs[k]["expl_out"] for k in range(NCORES)], 0)
    novelty = np.concatenate([rs[k]["nov_out"] for k in range(NCORES)], 0)
    place_cells = np.concatenate([rs[k]["pc_out"] for k in range(NCORES)], 0)
    new_map = np.concatenate([rs[k]["map_out"] for k in range(NCORES)], 0)[:M * M]
    new_visit = np.concatenate([rs[k]["visit_out"][:, 0] for k in range(NCORES)], 0)[:M * M]

    return (direction_probs, speed, expl, novelty, place_cells,
            new_map.reshape(M, M, D), new_visit.reshape(M, M))


def kernel(**inputs):
    global _NC_CACHE
    if _NC_CACHE is None:
        _NC_CACHE = build_kernel()
    in_maps = _make_in_maps(**inputs)
    res = run_bass_kernel_spmd(_NC_CACHE, in_maps, core_ids=list(range(NCORES)))
    return _gather_outputs(res.results)


def run_timed(inputs):
    """Re-run with NTFF tracing; returns exec_time_ns (requires hookreg import)."""
    global _NC_CACHE
    if _NC_CACHE is None:
        _NC_CACHE = build_kernel()
    in_maps = _make_in_maps(**inputs)
    res = run_bass_kernel_spmd(_NC_CACHE, in_maps, core_ids=list(range(NCORES)),
                               trace=True)
    return res.exec_time_ns
